# revision 2
# baseline (speedup 1.0000x reference)
"""Binary 3x3 conv (XNOR-net style) on 8 Trainium2 NeuronCores.

out = alpha * (sign(x) conv sign(w)), NHWC, SAME padding.
Data-parallel over batch: each of the 8 cores handles 8 images.

Per-core pipeline (all layout math hardcoded for x=(64,128,128,64) fp32):
  1. SWDGE cast-DMA image (fp32 HBM -> bf16 SBUF), row-major [row, w*64+ci],
     with 64-elem zero pads on both ends of each row.
  2. ACT Sign -> +-1 bf16 (exact in bf16).
  3. HWDGE xbar-transpose DMA -> "layout B": [k=channel-of-staggered-pixel-pair,
     (pair c, row r)] where k<64 is x[2c-1] channels, k>=64 is x[2c] channels.
  4. TensorE: 6 matmuls (K=128, M=128, N<=512) per 8 output rows, accumulating
     +-alpha contributions into one PSUM bank.  M packs (even-pixel cout |
     odd-pixel cout); weights are host-packed alpha*sign matrices with zero
     quadrants (alpha folded into the weights: bf16 alpha costs ~1e-3 rel,
     tolerance is 2e-2).
  5. ACT/DVE Copy evict PSUM fp32 -> fp16 SBUF (|alpha*count| <= ~60, rel
     5e-4), alternating engines to balance load.
  6. HWDGE xbar-transpose back to pixel-major fp16, split into 8 concurrent
     DMAs (concurrent xbar calls aggregate ~375 GB/s vs ~107 monolithic).
  7. SWDGE cast-DMA store fp16 SBUF -> fp32 HBM (halves SBUF-side reads).
"""

import os
import sys

sys.path.insert(0, "/opt/trn_rl_repo")

import numpy as np
import ml_dtypes

import concourse.bass as bass
import concourse.mybir as mybir
from concourse.tile import TileContext
from concourse.bass_utils import run_bass_kernel_spmd

N_CORES = 8
IMGS_PER_CORE = 8
H = W = 128
C = 64
ROW = W * C          # 8192 elems per image row
XPAD = 64            # one pixel of zero padding each side
XROW = ROW + 2 * XPAD  # 8320 = 65*128


def _split_multi_waits(nc):
    """The walrus in this container allows only ONE sync-wait per instruction.

    Tile attaches several waits to some instructions; hoist the extras onto
    single-wait NOPs inserted immediately before, on the same engine (the
    engine blocks on each in program order, so semantics are unchanged)."""
    n_new = 0
    for f in nc.m.functions:
        for bb in f.blocks:
            insts = bb.instructions
            if not any(
                i.sync_info is not None and len(i.sync_info.on_wait) > 1
                for i in insts
            ):
                continue
            new = []
            for inst in insts:
                si = inst.sync_info
                if si is not None and len(si.on_wait) > 1:
                    waits = list(si.on_wait)
                    for j, w in enumerate(waits[:-1]):
                        n_new += 1
                        new.append(mybir.InstNoOp(
                            name=f"{inst.name}-sw{j}",
                            engine=inst.engine,
                            bass_nofuse=True,
                            sync_info=mybir.SyncInfo(on_wait=[w], on_update=[]),
                        ))
                    si.on_wait.clear()
                    si.on_wait.append(waits[-1])
                new.append(inst)
            bb.instructions = new
    return n_new


def _pack_weights(w_fp: np.ndarray):
    """Host-side weight prep: alpha*sign matrices A/B per kh.

    alpha (per-cout mean |w|) is folded into the weight columns, so PSUM
    accumulates the final alpha-scaled output directly."""
    alpha = np.mean(np.abs(w_fp), axis=(0, 1, 2)).astype(np.float32)  # (co,)
    ws = np.where(w_fp >= 0, 1.0, -1.0).astype(np.float32) * alpha  # (kh,kw,ci,co)
    wst = np.zeros((6, 128, 128), np.float32)
    for kh in range(3):
        A = wst[2 * kh]
        B = wst[2 * kh + 1]
        # M columns: m<64 -> even out pixel w=2g cout m ; m>=64 -> odd w=2g+1.
        # K rows: k<64 -> x[2g-1] chan k ; k>=64 -> x[2g] chan k-64 (rhs pair g)
        # B variant reads pair g+1: k<64 -> x[2g+1], k>=64 -> x[2g+2].
        A[0:64, 0:64] = ws[kh, 0]
        A[64:128, 0:64] = ws[kh, 1]
        A[64:128, 64:128] = ws[kh, 0]
        B[0:64, 0:64] = ws[kh, 2]
        B[0:64, 64:128] = ws[kh, 1]
        B[64:128, 64:128] = ws[kh, 2]
    return np.ascontiguousarray(wst.astype(ml_dtypes.bfloat16))


_PROGRAM_CACHE = {}


def _build_program(repeats: int = 1, skip: tuple = ()):
    key = (repeats, tuple(sorted(skip)))
    if key in _PROGRAM_CACHE:
        return _PROGRAM_CACHE[key]
    skip = set(skip)

    f32 = mybir.dt.float32
    f16 = mybir.dt.float16
    bf16 = mybir.dt.bfloat16
    Copy = mybir.ActivationFunctionType.Copy

    nc = bass.Bass()
    x_d = nc.dram_tensor("x", (IMGS_PER_CORE, H, W, C), f32, kind="ExternalInput")
    wst_d = nc.dram_tensor("wst", (6, 128, 128), bf16, kind="ExternalInput")
    out_d = nc.dram_tensor("out", (IMGS_PER_CORE, H, W, C), f32, kind="ExternalOutput")

    x_flat = x_d.rearrange("i h w c -> i (h w c)")      # [8, 1048576]
    out_flat = out_d.rearrange("i h w c -> i (h w c)")  # [8, 1048576]

    with TileContext(nc) as tc:
        with (
            tc.tile_pool(name="wpool", bufs=1) as wpool,
            tc.tile_pool(name="xpool", bufs=2) as xpool,
            tc.tile_pool(name="xtpool", bufs=2) as xtpool,
            tc.tile_pool(name="ppool", bufs=8, space="PSUM") as ppool,
            tc.tile_pool(name="opool", bufs=3) as opool,
            tc.tile_pool(name="tpool", bufs=3) as tpool,
        ):
            wst_sb = wpool.tile([128, 6 * 128], bf16)
            nc.sync.dma_start(out=wst_sb.rearrange("k (i m) -> k i m", m=128),
                              in_=wst_d.rearrange("i k m -> k i m"))
            if skip:
                XCONST = wpool.tile([128, XROW], bf16)
                nc.vector.memset(XCONST[:, 0:XROW], 0.0)
                XTCONST = wpool.tile([128, XROW], bf16)
                nc.vector.memset(XTCONST[:, 0:XROW], 0.0)
                O4CONST = wpool.tile([128, 2048], f16)
                nc.vector.memset(O4CONST[:, 0:2048], 0.0)
                T2CONST = wpool.tile([128, 2048], f16)
                nc.vector.memset(T2CONST[:, 0:2048], 0.0)

            for img_rep in range(IMGS_PER_CORE * repeats):
                img = img_rep % IMGS_PER_CORE
                # --- load + sign + transpose to layout B ---
                if "cast" not in skip:
                    X = xpool.tile([128, XROW], bf16, tag="X")
                    nc.vector.memset(X[:, 0:XPAD], 0.0)
                    nc.vector.memset(X[:, XPAD + ROW:], 0.0)
                    nc.gpsimd.dma_start(
                        out=X[:, XPAD:XPAD + ROW],
                        in_=x_flat[img].rearrange("(h i) -> h i", h=128),
                    )
                    if "sign" not in skip:
                        nc.scalar.sign(X[:, XPAD:XPAD + ROW], X[:, XPAD:XPAD + ROW])
                else:
                    X = XCONST
                if "inxbar" not in skip:
                    XT = xtpool.tile([128, XROW], bf16, tag="XT")
                    XT3 = XT.rearrange("q (c r) -> q c r", r=128)
                    # split into 8 transposes: concurrent xbar DMAs aggregate to
                    # ~375 GB/s vs ~107 GB/s for one monolithic call (HW-measured)
                    for i8 in range(8):
                        c0 = i8 * 8
                        cn = 8 if i8 < 7 else 9
                        nc.sync.dma_start(
                            out=XT3[:, c0:c0 + cn, :],
                            in_=X[:, c0 * 128:(c0 + cn) * 128],
                            transpose=True,
                        )
                else:
                    XT = XTCONST
                # [128, c=65, r=128]; matmul rhs iterates (c outer, r inner) so the
                # innermost stream dim is stride-1 (16B runs) — strided innermost
                # dims run the PE ~3x slower (HW-measured).
                XT_cr = XT.rearrange("q (c r) -> q c r", r=128)

                # --- conv blocks: 16 blocks of 8 output rows; groups of 4 ---
                for grp in range(4):
                    if "mm" not in skip:
                        O4 = opool.tile([128, 2048], f16, tag="O4")
                    else:
                        O4 = O4CONST
                    for blk in range(4) if "mm" not in skip else []:
                        h0 = grp * 32 + blk * 8
                        psum = ppool.tile([128, 512], f32, tag="ps")
                        # same addresses (n = r*64 + c) but iterated (c outer,
                        # r inner) to pair with the rhs stream order
                        psum_cr = psum.rearrange("p (r c) -> p c r", c=64)
                        mms = []
                        for kh in (1, 0, 2):
                            rbase = h0 + kh - 1
                            r_lo = max(0, -rbase)
                            r_hi = min(8, 128 - rbase)
                            for v in (0, 1):  # A, B
                                mms.append((kh, v, rbase, r_lo, r_hi))
                        last = len(mms) - 1
                        for idx, (kh, v, rbase, r_lo, r_hi) in enumerate(mms):
                            lhsT = wst_sb[:, (2 * kh + v) * 128:(2 * kh + v + 1) * 128]
                            rhs = XT_cr[:, v:v + 64, rbase + r_lo:rbase + r_hi]
                            outp = psum_cr[:, :, r_lo:r_hi]
                            nc.tensor.matmul(outp, lhsT, rhs,
                                             start=(idx == 0), stop=(idx == last))
                        # evict PSUM fp32 -> fp16 SBUF; alternate ACT/DVE to
                        # balance engine load (alpha already in the weights)
                        oslice = O4[:, blk * 512:(blk + 1) * 512]
                        if blk % 2 == 0:
                            nc.scalar.activation(out=oslice, in_=psum[:], func=Copy)
                        else:
                            nc.vector.tensor_copy(oslice, psum[:])
                    # --- transpose back (8 concurrent xbars), cast-store ---
                    if "oxbar" not in skip:
                        T2 = tpool.tile([128, 2048], f16, tag="T2")
                        T23 = T2.rearrange("q (m p) -> q m p", p=128)
                        for i8 in range(8):
                            m0 = i8 * 2
                            nc.sync.dma_start(
                                out=T23[:, m0:m0 + 2, :],
                                in_=O4[:, m0 * 128:(m0 + 2) * 128],
                                transpose=True,
                            )
                    else:
                        T2 = T2CONST
                    if "store" not in skip:
                        dst = out_flat[img, grp * 262144:(grp + 1) * 262144]
                        # SWDGE ring: keeps the store traffic off the SP HWDGE
                        # ring where it serialized with the xbar transposes;
                        # fp16 -> fp32 cast on the fly (SWDGE-only feature).
                        nc.gpsimd.dma_start(
                            out=dst.rearrange("(m q p) -> q m p", m=16, q=128, p=128),
                            in_=T2.rearrange("q (m p) -> q m p", p=128),
                        )

    _split_multi_waits(nc)
    _PROGRAM_CACHE[key] = nc
    return nc


def _in_maps(x: np.ndarray, wst: np.ndarray):
    maps = []
    for i in range(N_CORES):
        maps.append({
            "x": x[i * IMGS_PER_CORE:(i + 1) * IMGS_PER_CORE],
            "wst": wst,
        })
    return maps


def kernel(x: np.ndarray, w_fp: np.ndarray) -> np.ndarray:
    assert x.shape == (64, 128, 128, 64) and w_fp.shape == (3, 3, 64, 64)
    x = np.ascontiguousarray(x, dtype=np.float32)
    wst = _pack_weights(np.asarray(w_fp, dtype=np.float32))

    nc = _build_program()
    res = run_bass_kernel_spmd(nc, _in_maps(x, wst), core_ids=list(range(N_CORES)))
    out = np.concatenate([r["out"] for r in res.results], axis=0)
    # stash perf info for test harnesses
    kernel.last_results = res
    return out


# revision 6
# speedup vs baseline: 1.2449x; 1.2449x over previous
"""Binary 3x3 conv (XNOR-net style) on 8 Trainium2 NeuronCores.

out = alpha * (sign(x) conv sign(w)), NHWC, SAME padding.
Data-parallel over batch: each of the 8 cores handles 8 images.

Per-core pipeline (all layout math hardcoded for x=(64,128,128,64) fp32):
  1. SWDGE cast-DMA image (fp32 HBM -> bf16 SBUF), row-major [row, w*64+ci],
     with 64-elem zero pads on both ends of each row.
  2. ACT Sign -> +-1 bf16 (exact in bf16).
  3. HWDGE xbar-transpose DMA -> "layout B": [k=channel-of-staggered-pixel-pair,
     (pair c, row r)] where k<64 is x[2c-1] channels, k>=64 is x[2c] channels.
  4. TensorE: 6 matmuls (K=128, M=128, N<=512) per 8 output rows, accumulating
     +-alpha contributions into one PSUM bank.  M packs (even-pixel cout |
     odd-pixel cout); weights are host-packed alpha*sign matrices with zero
     quadrants (alpha folded into the weights: bf16 alpha costs ~1e-3 rel,
     tolerance is 2e-2).
  5. ACT/DVE Copy evict PSUM fp32 -> fp16 SBUF (|alpha*count| <= ~60, rel
     5e-4), alternating engines to balance load.
  6. HWDGE xbar-transpose back to pixel-major fp16, split into 8 concurrent
     DMAs (concurrent xbar calls aggregate ~375 GB/s vs ~107 monolithic).
  7. SWDGE cast-DMA store fp16 SBUF -> fp32 HBM (halves SBUF-side reads).
"""

import os
import sys

sys.path.insert(0, "/opt/trn_rl_repo")

import numpy as np
import ml_dtypes

import concourse.bass as bass
import concourse.mybir as mybir
from concourse.tile import TileContext
from concourse.bass_utils import run_bass_kernel_spmd

N_CORES = 8
IMGS_PER_CORE = 8
H = W = 128
C = 64
ROW = W * C          # 8192 elems per image row
XPAD = 64            # one pixel of zero padding each side
XROW = ROW + 2 * XPAD  # 8320 = 65*128


def _split_multi_waits(nc):
    """The walrus in this container allows only ONE sync-wait per instruction.

    Tile attaches several waits to some instructions; hoist the extras onto
    single-wait NOPs inserted immediately before, on the same engine (the
    engine blocks on each in program order, so semantics are unchanged)."""
    n_new = 0
    for f in nc.m.functions:
        for bb in f.blocks:
            insts = bb.instructions
            if not any(
                i.sync_info is not None and len(i.sync_info.on_wait) > 1
                for i in insts
            ):
                continue
            new = []
            for inst in insts:
                si = inst.sync_info
                if si is not None and len(si.on_wait) > 1:
                    waits = list(si.on_wait)
                    for j, w in enumerate(waits[:-1]):
                        n_new += 1
                        new.append(mybir.InstNoOp(
                            name=f"{inst.name}-sw{j}",
                            engine=inst.engine,
                            bass_nofuse=True,
                            sync_info=mybir.SyncInfo(on_wait=[w], on_update=[]),
                        ))
                    si.on_wait.clear()
                    si.on_wait.append(waits[-1])
                new.append(inst)
            bb.instructions = new
    return n_new


def _pack_weights(w_fp: np.ndarray):
    """Host-side weight prep: alpha*sign matrices A/B per kh.

    alpha (per-cout mean |w|) is folded into the weight columns, so PSUM
    accumulates the final alpha-scaled output directly."""
    alpha = np.mean(np.abs(w_fp), axis=(0, 1, 2)).astype(np.float32)  # (co,)
    ws = np.where(w_fp >= 0, 1.0, -1.0).astype(np.float32) * alpha  # (kh,kw,ci,co)
    wst = np.zeros((6, 128, 128), np.float32)
    for kh in range(3):
        A = wst[2 * kh]
        B = wst[2 * kh + 1]
        # M columns: m<64 -> even out pixel w=2g cout m ; m>=64 -> odd w=2g+1.
        # K rows: k<64 -> x[2g-1] chan k ; k>=64 -> x[2g] chan k-64 (rhs pair g)
        # B variant reads pair g+1: k<64 -> x[2g+1], k>=64 -> x[2g+2].
        A[0:64, 0:64] = ws[kh, 0]
        A[64:128, 0:64] = ws[kh, 1]
        A[64:128, 64:128] = ws[kh, 0]
        B[0:64, 0:64] = ws[kh, 2]
        B[0:64, 64:128] = ws[kh, 1]
        B[64:128, 64:128] = ws[kh, 2]
    return np.ascontiguousarray(wst.astype(ml_dtypes.bfloat16))


_PROGRAM_CACHE = {}


def _build_program(repeats: int = 1, skip: tuple = ()):
    key = (repeats, tuple(sorted(skip)))
    if key in _PROGRAM_CACHE:
        return _PROGRAM_CACHE[key]
    skip = set(skip)

    f32 = mybir.dt.float32
    f16 = mybir.dt.float16
    bf16 = mybir.dt.bfloat16
    Copy = mybir.ActivationFunctionType.Copy

    nc = bass.Bass()
    x_d = nc.dram_tensor("x", (IMGS_PER_CORE, H, W, C), f32, kind="ExternalInput")
    wst_d = nc.dram_tensor("wst", (6, 128, 128), bf16, kind="ExternalInput")
    out_d = nc.dram_tensor("out", (IMGS_PER_CORE, H, W, C), f32, kind="ExternalOutput")

    x_flat = x_d.rearrange("i h w c -> i (h w c)")      # [8, 1048576]
    out_flat = out_d.rearrange("i h w c -> i (h w c)")  # [8, 1048576]

    with TileContext(nc) as tc:
        with (
            tc.tile_pool(name="wpool", bufs=1) as wpool,
            tc.tile_pool(name="xpool", bufs=2) as xpool,
            tc.tile_pool(name="xtpool", bufs=2) as xtpool,
            tc.tile_pool(name="ppool", bufs=8, space="PSUM") as ppool,
            tc.tile_pool(name="opool", bufs=3) as opool,
            tc.tile_pool(name="tpool", bufs=3) as tpool,
        ):
            wst_sb = wpool.tile([128, 6 * 128], bf16)
            nc.sync.dma_start(out=wst_sb.rearrange("k (i m) -> k i m", m=128),
                              in_=wst_d.rearrange("i k m -> k i m"))
            if skip:
                XCONST = wpool.tile([128, XROW], bf16)
                nc.vector.memset(XCONST[:, 0:XROW], 0.0)
                XTCONST = wpool.tile([128, XROW], bf16)
                nc.vector.memset(XTCONST[:, 0:XROW], 0.0)
                O4CONST = wpool.tile([128, 2048], f16)
                nc.vector.memset(O4CONST[:, 0:2048], 0.0)
                T2CONST = wpool.tile([128, 2048], f16)
                nc.vector.memset(T2CONST[:, 0:2048], 0.0)

            for img_rep in range(IMGS_PER_CORE * repeats):
                img = img_rep % IMGS_PER_CORE
                # --- load + sign + transpose to layout B ---
                if "cast" not in skip:
                    X = xpool.tile([128, XROW], bf16, tag="X")
                    nc.vector.memset(X[:, 0:XPAD], 0.0)
                    nc.vector.memset(X[:, XPAD + ROW:], 0.0)
                    # halves: finer-grained load->sign->xbar pipelining
                    xv = x_flat[img].rearrange("(h i) -> h i", h=128)
                    nc.gpsimd.dma_start(out=X[:, 64:4224], in_=xv[:, 0:4160])
                    nc.gpsimd.dma_start(out=X[:, 4224:8256], in_=xv[:, 4160:8192])
                    if "sign" not in skip:
                        nc.scalar.sign(X[:, 64:4224], X[:, 64:4224])
                        nc.scalar.sign(X[:, 4224:8256], X[:, 4224:8256])
                else:
                    X = XCONST
                if "inxbar" not in skip:
                    XT = xtpool.tile([128, XROW], bf16, tag="XT")
                    XT3 = XT.rearrange("q (c r) -> q c r", r=128)
                    # split into 8 transposes: concurrent xbar DMAs aggregate to
                    # ~375 GB/s vs ~107 GB/s for one monolithic call (HW-measured)
                    for i8 in range(8):
                        c0 = i8 * 8
                        cn = 8 if i8 < 7 else 9
                        nc.sync.dma_start(
                            out=XT3[:, c0:c0 + cn, :],
                            in_=X[:, c0 * 128:(c0 + cn) * 128],
                            transpose=True,
                        )
                else:
                    XT = XTCONST
                # [128, c=65, r=128]; matmul rhs iterates (c outer, r inner) so the
                # innermost stream dim is stride-1 (16B runs) — strided innermost
                # dims run the PE ~3x slower (HW-measured).
                XT_cr = XT.rearrange("q (c r) -> q c r", r=128)

                # --- conv blocks: 16 blocks of 8 output rows; groups of 4 ---
                for grp in range(4):
                    if "mm" not in skip:
                        O4 = opool.tile([128, 2048], f16, tag="O4")
                    else:
                        O4 = O4CONST
                    for blk in range(4) if "mm" not in skip else []:
                        h0 = grp * 32 + blk * 8
                        psum = ppool.tile([128, 512], f32, tag="ps")
                        # same addresses (n = r*64 + c) but iterated (c outer,
                        # r inner) to pair with the rhs stream order
                        psum_cr = psum.rearrange("p (r c) -> p c r", c=64)
                        mms = []
                        for kh in (1, 0, 2):
                            rbase = h0 + kh - 1
                            r_lo = max(0, -rbase)
                            r_hi = min(8, 128 - rbase)
                            for v in (0, 1):  # A, B
                                mms.append((kh, v, rbase, r_lo, r_hi))
                        last = len(mms) - 1
                        for idx, (kh, v, rbase, r_lo, r_hi) in enumerate(mms):
                            lhsT = wst_sb[:, (2 * kh + v) * 128:(2 * kh + v + 1) * 128]
                            rhs = XT_cr[:, v:v + 64, rbase + r_lo:rbase + r_hi]
                            outp = psum_cr[:, :, r_lo:r_hi]
                            nc.tensor.matmul(outp, lhsT, rhs,
                                             start=(idx == 0), stop=(idx == last))
                        # evict PSUM fp32 -> fp16 SBUF; alternate ACT/DVE to
                        # balance engine load (alpha already in the weights)
                        oslice = O4[:, blk * 512:(blk + 1) * 512]
                        if blk % 2 == 0:
                            nc.scalar.activation(out=oslice, in_=psum[:], func=Copy)
                        else:
                            nc.vector.tensor_copy(oslice, psum[:])
                    # --- transpose back (8 concurrent xbars), cast-store ---
                    if "oxbar" not in skip:
                        T2 = tpool.tile([128, 2048], f16, tag="T2")
                        T23 = T2.rearrange("q (m p) -> q m p", p=128)
                        # 2 calls of 8 tiles (256 KB) — same chunk size that
                        # works for the input xbar; 64 KB calls were fixed-
                        # cost dominated (HW-measured +100us marginal)
                        for i2 in range(2):
                            m0 = i2 * 8
                            nc.sync.dma_start(
                                out=T23[:, m0:m0 + 8, :],
                                in_=O4[:, m0 * 128:(m0 + 8) * 128],
                                transpose=True,
                            )
                    else:
                        T2 = T2CONST
                    if "store" not in skip:
                        dst = out_flat[img, grp * 262144:(grp + 1) * 262144]
                        # SWDGE ring: keeps the store traffic off the SP HWDGE
                        # ring where it serialized with the xbar transposes;
                        # fp16 -> fp32 cast on the fly (SWDGE-only feature).
                        nc.gpsimd.dma_start(
                            out=dst.rearrange("(m q p) -> q m p", m=16, q=128, p=128),
                            in_=T2.rearrange("q (m p) -> q m p", p=128),
                        )

    _split_multi_waits(nc)
    _PROGRAM_CACHE[key] = nc
    return nc


def _in_maps(x: np.ndarray, wst: np.ndarray):
    maps = []
    for i in range(N_CORES):
        maps.append({
            "x": x[i * IMGS_PER_CORE:(i + 1) * IMGS_PER_CORE],
            "wst": wst,
        })
    return maps


def _pack_inputs(x, w_fp):
    x = np.ascontiguousarray(x, dtype=np.float32)
    wst = _pack_weights(np.asarray(w_fp, dtype=np.float32))
    return _in_maps(x, wst)


def kernel(x: np.ndarray, w_fp: np.ndarray) -> np.ndarray:
    assert x.shape == (64, 128, 128, 64) and w_fp.shape == (3, 3, 64, 64)

    nc = _build_program()
    res = run_bass_kernel_spmd(nc, _pack_inputs(x, w_fp), core_ids=list(range(N_CORES)))
    out = np.concatenate([r["out"] for r in res.results], axis=0)
    # stash perf info for test harnesses
    kernel.last_results = res
    return out


# revision 7
# speedup vs baseline: 2.6710x; 2.1455x over previous
"""Binary 3x3 conv (XNOR-net style) on 8 Trainium2 NeuronCores.

out = alpha * (sign(x) conv sign(w)), NHWC, SAME padding.
Data-parallel over batch: each of the 8 cores handles 8 images.

Per-core pipeline (all layout math hardcoded for x=(64,128,128,64) fp32):
  1. SWDGE cast-DMA image (fp32 HBM -> bf16 SBUF), row-major [row, w*64+ci],
     with 64-elem zero pads on both ends of each row.
  2. ACT Sign -> +-1 bf16 (exact in bf16).
  3. HWDGE xbar-transpose DMA -> "layout B": [k=channel-of-staggered-pixel-pair,
     (pair c, row r)] where k<64 is x[2c-1] channels, k>=64 is x[2c] channels.
  4. TensorE: 6 matmuls (K=128, M=128, N<=512) per 8 output rows, accumulating
     integer +-1 counts into one PSUM bank.  M packs (even-pixel cout | odd-pixel
     cout); weights are host-packed sign matrices with zero quadrants.
  5. ACT Copy evict PSUM fp32 counts -> fp16 SBUF (counts <= 576, exact).
  6. HWDGE xbar-transpose back to pixel-major fp16.
  7. DVE tensor_mul with replicated fp32 alpha -> fp32.
  8. Contiguous DMA store.
"""

import os
import sys

sys.path.insert(0, "/opt/trn_rl_repo")

import numpy as np
import ml_dtypes

import concourse.bass as bass
import concourse.mybir as mybir
from concourse.tile import TileContext
from concourse.bass_utils import run_bass_kernel_spmd

N_CORES = 8
IMGS_PER_CORE = 8
H = W = 128
C = 64
ROW = W * C          # 8192 elems per image row
XPAD = 64            # one pixel of zero padding each side
XROW = ROW + 2 * XPAD  # 8320 = 65*128


def _split_multi_waits(nc):
    """The walrus in this container allows only ONE sync-wait per instruction.

    Tile attaches several waits to some instructions; hoist the extras onto
    single-wait NOPs inserted immediately before, on the same engine (the
    engine blocks on each in program order, so semantics are unchanged)."""
    n_new = 0
    for f in nc.m.functions:
        for bb in f.blocks:
            insts = bb.instructions
            if not any(
                i.sync_info is not None and len(i.sync_info.on_wait) > 1
                for i in insts
            ):
                continue
            new = []
            for inst in insts:
                si = inst.sync_info
                if si is not None and len(si.on_wait) > 1:
                    waits = list(si.on_wait)
                    for j, w in enumerate(waits[:-1]):
                        n_new += 1
                        new.append(mybir.InstNoOp(
                            name=f"{inst.name}-sw{j}",
                            engine=inst.engine,
                            bass_nofuse=True,
                            sync_info=mybir.SyncInfo(on_wait=[w], on_update=[]),
                        ))
                    si.on_wait.clear()
                    si.on_wait.append(waits[-1])
                new.append(inst)
            bb.instructions = new
    return n_new


def _pack_weights(w_fp: np.ndarray):
    """Host-side weight prep: sign matrices A/B per kh, and alpha replication."""
    ws = np.where(w_fp >= 0, 1.0, -1.0).astype(np.float32)  # (kh, kw, ci, co)
    wst = np.zeros((6, 128, 128), np.float32)
    for kh in range(3):
        A = wst[2 * kh]
        B = wst[2 * kh + 1]
        # M columns: m<64 -> even out pixel w=2g cout m ; m>=64 -> odd w=2g+1.
        # K rows: k<64 -> x[2g-1] chan k ; k>=64 -> x[2g] chan k-64 (rhs pair g)
        # B variant reads pair g+1: k<64 -> x[2g+1], k>=64 -> x[2g+2].
        A[0:64, 0:64] = ws[kh, 0]
        A[64:128, 0:64] = ws[kh, 1]
        A[64:128, 64:128] = ws[kh, 0]
        B[0:64, 0:64] = ws[kh, 2]
        B[0:64, 64:128] = ws[kh, 1]
        B[64:128, 64:128] = ws[kh, 2]
    alpha = np.mean(np.abs(w_fp), axis=(0, 1, 2)).astype(np.float32)  # (co,)
    alpha_rep = np.tile(alpha, 32)[None, :].repeat(128, axis=0)  # (128, 2048)
    return wst.astype(ml_dtypes.bfloat16), np.ascontiguousarray(alpha_rep)


_PROGRAM_CACHE = {}


def _build_program(repeats: int = 1, skip: tuple = ()):
    key = (repeats, tuple(sorted(skip)))
    if key in _PROGRAM_CACHE:
        return _PROGRAM_CACHE[key]
    skip = set(skip)

    f32 = mybir.dt.float32
    f16 = mybir.dt.float16
    bf16 = mybir.dt.bfloat16
    Copy = mybir.ActivationFunctionType.Copy

    nc = bass.Bass()
    x_d = nc.dram_tensor("x", (IMGS_PER_CORE, H, W, C), f32, kind="ExternalInput")
    wst_d = nc.dram_tensor("wst", (6, 128, 128), bf16, kind="ExternalInput")
    al_d = nc.dram_tensor("alpha_rep", (128, 2048), f32, kind="ExternalInput")
    out_d = nc.dram_tensor("out", (IMGS_PER_CORE, H, W, C), f32, kind="ExternalOutput")

    x_flat = x_d.rearrange("i h w c -> i (h w c)")      # [8, 1048576]
    out_flat = out_d.rearrange("i h w c -> i (h w c)")  # [8, 1048576]

    with TileContext(nc) as tc:
        with (
            tc.tile_pool(name="wpool", bufs=1) as wpool,
            tc.tile_pool(name="xpool", bufs=2) as xpool,
            tc.tile_pool(name="xtpool", bufs=2) as xtpool,
            tc.tile_pool(name="ppool", bufs=8, space="PSUM") as ppool,
            tc.tile_pool(name="opool", bufs=3) as opool,
            tc.tile_pool(name="tpool", bufs=3) as tpool,
            tc.tile_pool(name="fpool", bufs=3) as fpool,
        ):
            wst_sb = wpool.tile([128, 6 * 128], bf16)
            nc.sync.dma_start(out=wst_sb.rearrange("k (i m) -> k i m", m=128),
                              in_=wst_d.rearrange("i k m -> k i m"))
            alpha_sb = wpool.tile([128, 2048], f32)
            nc.sync.dma_start(out=alpha_sb[:], in_=al_d[:])
            if skip:
                XCONST = wpool.tile([128, XROW], bf16)
                nc.vector.memset(XCONST[:, 0:XROW], 0.0)
                XTCONST = wpool.tile([128, XROW], bf16)
                nc.vector.memset(XTCONST[:, 0:XROW], 0.0)
                O4CONST = wpool.tile([128, 2048], f16)
                nc.vector.memset(O4CONST[:, 0:2048], 0.0)
                T2CONST = wpool.tile([128, 2048], f16)
                nc.vector.memset(T2CONST[:, 0:2048], 0.0)
                F32CONST = wpool.tile([128, 2048], f32)
                nc.vector.memset(F32CONST[:, 0:2048], 0.0)

            for img_rep in range(IMGS_PER_CORE * repeats):
                img = img_rep % IMGS_PER_CORE
                # --- load + sign + transpose to layout B ---
                if "cast" not in skip:
                    X = xpool.tile([128, XROW], bf16, tag="X")
                    nc.vector.memset(X[:, 0:XPAD], 0.0)
                    nc.vector.memset(X[:, XPAD + ROW:], 0.0)
                    nc.gpsimd.dma_start(
                        out=X[:, XPAD:XPAD + ROW],
                        in_=x_flat[img].rearrange("(h i) -> h i", h=128),
                    )
                    if "sign" not in skip:
                        nc.scalar.sign(X[:, XPAD:XPAD + ROW], X[:, XPAD:XPAD + ROW])
                else:
                    X = XCONST
                if "inxbar" not in skip:
                    XT = xtpool.tile([128, XROW], bf16, tag="XT")
                    XT3 = XT.rearrange("q (c r) -> q c r", r=128)
                    # split into 8 transposes: concurrent xbar DMAs aggregate to
                    # ~375 GB/s vs ~107 GB/s for one monolithic call (HW-measured)
                    for i8 in range(8):
                        c0 = i8 * 8
                        cn = 8 if i8 < 7 else 9
                        nc.sync.dma_start(
                            out=XT3[:, c0:c0 + cn, :],
                            in_=X[:, c0 * 128:(c0 + cn) * 128],
                            transpose=True,
                        )
                else:
                    XT = XTCONST
                # [128, c=65, r=128]; matmul rhs iterates (c outer, r inner) so the
                # innermost stream dim is stride-1 (16B runs) — strided innermost
                # dims run the PE ~3x slower (HW-measured).
                XT_cr = XT.rearrange("q (c r) -> q c r", r=128)

                # --- conv blocks: 16 blocks of 8 output rows; groups of 4 ---
                for grp in range(4):
                    if "mm" not in skip:
                        O4 = opool.tile([128, 2048], f16, tag="O4")
                    else:
                        O4 = O4CONST
                    for blk in range(4) if "mm" not in skip else []:
                        h0 = grp * 32 + blk * 8
                        psum = ppool.tile([128, 512], f32, tag="ps")
                        # same addresses (n = r*64 + c) but iterated (c outer,
                        # r inner) to pair with the rhs stream order
                        psum_cr = psum.rearrange("p (r c) -> p c r", c=64)
                        mms = []
                        for kh in (1, 0, 2):
                            rbase = h0 + kh - 1
                            r_lo = max(0, -rbase)
                            r_hi = min(8, 128 - rbase)
                            for v in (0, 1):  # A, B
                                mms.append((kh, v, rbase, r_lo, r_hi))
                        last = len(mms) - 1
                        for idx, (kh, v, rbase, r_lo, r_hi) in enumerate(mms):
                            lhsT = wst_sb[:, (2 * kh + v) * 128:(2 * kh + v + 1) * 128]
                            rhs = XT_cr[:, v:v + 64, rbase + r_lo:rbase + r_hi]
                            outp = psum_cr[:, :, r_lo:r_hi]
                            nc.tensor.matmul(outp, lhsT, rhs,
                                             start=(idx == 0), stop=(idx == last))
                        nc.scalar.activation(out=O4[:, blk * 512:(blk + 1) * 512],
                                             in_=psum[:], func=Copy)
                    # --- transpose back, scale, store (32 rows = 262144 elems) ---
                    if "oxbar" not in skip:
                        T2 = tpool.tile([128, 2048], f16, tag="T2")
                        nc.sync.dma_start(
                            out=T2.rearrange("q (m p) -> q m p", p=128),
                            in_=O4[:],
                            transpose=True,
                        )
                    else:
                        T2 = T2CONST
                    if "tt" not in skip:
                        F32 = fpool.tile([128, 2048], f32, tag="F32")
                        nc.vector.tensor_mul(out=F32[:], in0=T2[:], in1=alpha_sb[:])
                    else:
                        F32 = F32CONST
                    if "store" not in skip:
                        dst = out_flat[img, grp * 262144:(grp + 1) * 262144]
                        # SWDGE ring: keeps the 128us of store traffic off the
                        # SP HWDGE ring where it serialized with the xbar
                        # transposes (safe: SBUF->DRAM copy, no xbar mode).
                        nc.gpsimd.dma_start(
                            out=dst.rearrange("(m q p) -> q m p", m=16, q=128, p=128),
                            in_=F32.rearrange("q (m p) -> q m p", p=128),
                        )

    _split_multi_waits(nc)
    _PROGRAM_CACHE[key] = nc
    return nc


def _in_maps(x: np.ndarray, wst: np.ndarray, alpha_rep: np.ndarray):
    maps = []
    for i in range(N_CORES):
        maps.append({
            "x": x[i * IMGS_PER_CORE:(i + 1) * IMGS_PER_CORE],
            "wst": wst,
            "alpha_rep": alpha_rep,
        })
    return maps


def _pack_inputs(x, w_fp):
    x = np.ascontiguousarray(x, dtype=np.float32)
    wst, alpha_rep = _pack_weights(np.asarray(w_fp, dtype=np.float32))
    return _in_maps(x, wst, alpha_rep)


def kernel(x: np.ndarray, w_fp: np.ndarray) -> np.ndarray:
    assert x.shape == (64, 128, 128, 64) and w_fp.shape == (3, 3, 64, 64)
    nc = _build_program()
    res = run_bass_kernel_spmd(nc, _pack_inputs(x, w_fp), core_ids=list(range(N_CORES)))
    out = np.concatenate([r["out"] for r in res.results], axis=0)
    # stash perf info for test harnesses
    kernel.last_results = res
    return out


# revision 8
# speedup vs baseline: 2.7160x; 1.0169x over previous
"""Binary 3x3 conv v4: PE-mode transposes + fp8 DoubleRow matmuls.

out = alpha * (sign(x) conv sign(w)), NHWC, SAME padding.
Data-parallel over batch: each of the 8 cores handles 8 images.

Design: keep the SDMA fabric at the pure HBM floor (4 MiB read + 4 MiB
write per image) by doing BOTH layout transposes on the TensorEngine
(128x128 transpose-mode tiles pipeline at ~81-107 ns), and halving the
conv's PE time with fp8e4 DoubleRow matmuls (virtual K=256; the k-tile
pair strides 128 B through the channel-major tile, which the 3D rhs AP
expresses directly).

Per-core pipeline per image:
  1. SWDGE cast-DMA (fp32 HBM -> bf16 SBUF) row-major X [r=128, (pp,ci)]
     with 1 pad pixel left (pp=0), 1 right (pp=129); pads NOT zeroed here
     (sign(0)=+1 would corrupt them; they are zeroed post-sign in XT8).
  2. PE transpose-mode, 65 tiles [128,128]: X tile c (pixels pp=2c,2c+1)
     -> PSUM bf16 [q=(wlo,ci), r].  8 tiles per PSUM bank.
  3. ACT Sign evict PSUM -> XT8 fp8 [q, (c,r)]; DVE memsets zero the two
     pad half-tiles (q<64 of tile 0, q>=64 of tile 64).
  4. fp8 DoubleRow conv: per 16-row x 64-col output block, 3 matmuls
     (kh in 1,0,2), each lhsT=[128,2,128] w8[kh], rhs=[128,(t=2),(c'=32),
     (r<=16)] with t,c' both striding 128 -> psum fp32 [m=(par,co),
     (c',r)].  Outputs w'=2(c0+c')+par; 75% PE utilization.
  5. DVE tensor_scalar_mul evict: psum * alpha[m%64] -> fp16 O4.
  6. PE transpose-mode 16 tiles/grp: O4 -> PSUM fp16; ACT Copy evict ->
     T2 fp16 pixel-major.
  7. SWDGE cast-DMA store fp16 -> fp32 HBM (512 B HBM runs).
"""

import os
import sys

sys.path.insert(0, "/opt/trn_rl_repo")

import numpy as np
import ml_dtypes

import concourse.bass as bass
import concourse.mybir as mybir
from concourse.bass_types import AP
from concourse import masks
from concourse.tile import TileContext
from concourse.bass_utils import run_bass_kernel_spmd

N_CORES = 8
IMGS_PER_CORE = 8
H = W = 128
C = 64
ROW = W * C           # 8192 bf16 per image row (data)
XROW = ROW + 128      # 8320 = 65*128: 1 pad pixel (64ch) each side
NT = 65               # transpose tiles per image


def _split_multi_waits(nc):
    """Single-sync-wait walrus workaround (see baseline)."""
    n_new = 0
    for f in nc.m.functions:
        for bb in f.blocks:
            insts = bb.instructions
            if not any(
                i.sync_info is not None and len(i.sync_info.on_wait) > 1
                for i in insts
            ):
                continue
            new = []
            for inst in insts:
                si = inst.sync_info
                if si is not None and len(si.on_wait) > 1:
                    waits = list(si.on_wait)
                    for j, w in enumerate(waits[:-1]):
                        n_new += 1
                        new.append(mybir.InstNoOp(
                            name=f"{inst.name}-sw{j}",
                            engine=inst.engine,
                            bass_nofuse=True,
                            sync_info=mybir.SyncInfo(on_wait=[w], on_update=[]),
                        ))
                    si.on_wait.clear()
                    si.on_wait.append(waits[-1])
                new.append(inst)
            bb.instructions = new
    return n_new


def _pack_weights(w_fp: np.ndarray):
    """Host prep: fp8 sign weights for DoubleRow + fp32 alpha column.

    w8[kh][64*wlo+ci, t, m]: m<64 -> kw=2t+wlo (if <=2); m>=64 -> kw=
    2t+wlo-1 (if >=0); else 0.  alpha_col[m] = alpha[m % 64]."""
    alpha = np.mean(np.abs(w_fp), axis=(0, 1, 2)).astype(np.float32)  # (co,)
    s = np.where(w_fp >= 0, 1.0, -1.0).astype(np.float32)  # (kh,kw,ci,co)
    w8 = np.zeros((3, 128, 2, 128), np.float32)
    for kh in range(3):
        for wlo in range(2):
            for t in range(2):
                kw_e = 2 * t + wlo        # even outputs (par=0, m<64)
                if kw_e <= 2:
                    w8[kh, 64 * wlo:64 * wlo + 64, t, 0:64] = s[kh, kw_e]
                kw_o = 2 * t + wlo - 1    # odd outputs (par=1, m>=64)
                if kw_o >= 0:
                    w8[kh, 64 * wlo:64 * wlo + 64, t, 64:128] = s[kh, kw_o]
    w8 = w8.astype(ml_dtypes.float8_e4m3)
    alpha_col = np.tile(alpha, 2)[:, None].astype(np.float32)  # (128, 1)
    return np.ascontiguousarray(w8), np.ascontiguousarray(alpha_col)


_PROGRAM_CACHE = {}


def _build_program(repeats: int = 1, skip: tuple = ()):
    key = (repeats, tuple(sorted(skip)))
    if key in _PROGRAM_CACHE:
        return _PROGRAM_CACHE[key]
    skip = set(skip)

    f32 = mybir.dt.float32
    f16 = mybir.dt.float16
    bf16 = mybir.dt.bfloat16
    fp8 = mybir.dt.float8e4
    Copy = mybir.ActivationFunctionType.Copy
    DR = mybir.MatmulPerfMode.DoubleRow

    nc = bass.Bass()
    x_d = nc.dram_tensor("x", (IMGS_PER_CORE, H, W, C), f32, kind="ExternalInput")
    w8_d = nc.dram_tensor("w8", (3, 128, 2, 128), fp8, kind="ExternalInput")
    al_d = nc.dram_tensor("alpha_col", (128, 1), f32, kind="ExternalInput")
    out_d = nc.dram_tensor("out", (IMGS_PER_CORE, H, W, C), f32, kind="ExternalOutput")

    x_flat = x_d.rearrange("i h w c -> i (h w c)")      # [8, 1048576]
    out_flat = out_d.rearrange("i h w c -> i (h w c)")  # [8, 1048576]

    with TileContext(nc) as tc:
        with (
            tc.tile_pool(name="wpool", bufs=1) as wpool,
            tc.tile_pool(name="xpool", bufs=2) as xpool,
            tc.tile_pool(name="x8pool", bufs=2) as x8pool,
            tc.tile_pool(name="tps", bufs=2, space="PSUM") as tps,
            tc.tile_pool(name="cps", bufs=4, space="PSUM") as cps,
            tc.tile_pool(name="ops", bufs=2, space="PSUM") as ops,
            tc.tile_pool(name="opool", bufs=3) as opool,
            tc.tile_pool(name="tpool", bufs=3) as tpool,
        ):
            w8_sb = wpool.tile([128, 768], fp8)
            nc.sync.dma_start(
                out=w8_sb.rearrange("k (i t m) -> k i t m", i=3, t=2),
                in_=w8_d.rearrange("i k t m -> k i t m"))
            w8v = w8_sb.rearrange("k (i t m) -> k i t m", i=3, t=2)
            alpha_sb = wpool.tile([128, 1], f32)
            nc.sync.dma_start(out=alpha_sb[:], in_=al_d[:])
            identb = wpool.tile([128, 128], bf16)
            masks.make_identity(nc, identb[:])
            identh = wpool.tile([128, 128], f16)
            masks.make_identity(nc, identh[:])
            if skip:
                XCONST = wpool.tile([128, XROW], bf16)
                nc.vector.memset(XCONST[:, :], 0.0)
                X8CONST = wpool.tile([128, XROW], fp8)
                nc.vector.memset(X8CONST[:, :], 0.0)
                O4CONST = wpool.tile([128, 2048], f16)
                nc.vector.memset(O4CONST[:, :], 0.0)
                T2CONST = wpool.tile([128, 2048], f16)
                nc.vector.memset(T2CONST[:, :], 0.0)

            def emit_load(img):
                """1. load (cast fp32 -> bf16), memset pads, ACT Sign in
                place -> +-1 bf16 with zero pads (sign runs BEFORE the
                transposes, so the transpose chain's evicts are plain
                copies splittable across DVE and ACT)."""
                if "cast" in skip:
                    return XCONST
                X = xpool.tile([128, XROW], bf16, tag="X")
                nc.vector.memset(X[:, 0:64], 0.0)
                nc.vector.memset(X[:, 64 + ROW:], 0.0)
                nc.gpsimd.dma_start(
                    out=X[:, 64:64 + ROW],
                    in_=x_flat[img].rearrange("(h i) -> h i", h=128),
                )
                nc.scalar.sign(X[:, 64:64 + ROW], X[:, 64:64 + ROW])
                return X

            def make_intr_chunks(X):
                """2+3. closures for bf16 PE-transpose chunks (8 tiles per
                PSUM bank) + copy-evicts (bf16 psum -> fp8 SBUF) alternating
                DVE/ACT.

                Returned as closures so the caller can interleave them with
                the previous image's matmuls (transpose-mode doesn't count
                as PE-busy for HAM; a 65-tile stretch would re-throttle the
                clock to 1.2 GHz)."""
                if "intr" in skip:
                    return X8CONST, []
                XT8 = x8pool.tile([128, XROW], fp8, tag="XT8")

                def chunk(j):
                    def go():
                        c1 = min(8 * j + 8, NT)
                        n = (c1 - 8 * j) * 128
                        pst = tps.tile([128, 1024], bf16, tag="pst")
                        for i, c in enumerate(range(8 * j, c1)):
                            nc.tensor.transpose(
                                pst[:, 128 * i:128 * i + 128],
                                X[:, 128 * c:128 * c + 128], identb[:])
                        dst = XT8[:, 1024 * j:1024 * j + n]
                        if j % 2 == 0:
                            nc.vector.tensor_copy(dst, pst[:, 0:n])
                        else:
                            nc.scalar.activation(out=dst, in_=pst[:, 0:n],
                                                 func=Copy)
                    return go

                return XT8, [chunk(j) for j in range(9)]

            def emit_conv(img, XT8, next_chunks):
                """4-7. conv + out-transpose + store, interleaving the next
                image's transpose chunks between groups."""
                xt8_ap = XT8[:]
                pstride = xt8_ap.ap[0][0]

                def dr_rhs(free_off, cnt):
                    return AP(xt8_ap.tensor, xt8_ap.offset + free_off,
                              [[pstride, 128], [128, 2], [128, 64], [1, cnt]])

                # psum memory layout n = r*64 + c' (c' innermost) so the
                # out-transpose q = (rlo, c') maps to a uniform HBM stride
                # (3-dim store AP; >3 dims unbalanceable).
                for grp in range(4):
                    # interleave 2-3 next-image transpose chunks per grp
                    lo = (9 * grp) // 4
                    hi = (9 * (grp + 1)) // 4
                    for j in range(lo, min(hi, len(next_chunks))):
                        next_chunks[j]()
                    if "mm" not in skip:
                        O4 = opool.tile([128, 2048], f16, tag="O4")
                        for blk in range(4):
                            h0 = grp * 32 + blk * 8
                            psum = cps.tile([128, 512], f32, tag="ps")
                            psv = psum.rearrange("m (r c) -> m c r", c=64)
                            for idx, kh in enumerate((1, 0, 2)):
                                rbase = h0 + kh - 1
                                r_lo = max(0, -rbase)
                                r_hi = min(8, 128 - rbase)
                                rhs = dr_rhs(rbase + r_lo, r_hi - r_lo)
                                nc.tensor.matmul(
                                    psv[:, :, r_lo:r_hi], w8v[:, kh], rhs,
                                    start=(idx == 0), stop=(idx == 2),
                                    perf_mode=DR)
                            nc.vector.tensor_scalar_mul(
                                O4[:, 512 * blk:512 * blk + 512],
                                psum[:], alpha_sb[:, 0:1])
                    else:
                        O4 = O4CONST
                    # --- 6. out-transpose 16 tiles + ACT evict ---
                    if "otr" not in skip:
                        T2 = tpool.tile([128, 2048], f16, tag="T2")
                        for half in range(2):
                            pso = ops.tile([128, 1024], f16, tag="pso")
                            for i in range(8):
                                m16 = 8 * half + i
                                nc.tensor.transpose(
                                    pso[:, 128 * i:128 * i + 128],
                                    O4[:, 128 * m16:128 * m16 + 128],
                                    identh[:])
                            nc.scalar.activation(
                                out=T2[:, 1024 * half:1024 * half + 1024],
                                in_=pso[:], func=Copy)
                    else:
                        T2 = T2CONST
                    # --- 7. cast-store fp16 -> fp32 (512 B HBM runs) ---
                    if "store" not in skip:
                        dst = out_flat[img, grp * 262144:(grp + 1) * 262144]
                        nc.gpsimd.dma_start(
                            out=dst.rearrange("(m q p) -> q m p",
                                              m=16, q=128, p=128),
                            in_=T2.rearrange("q (m p) -> q m p", p=128),
                        )

            # --- software pipeline: img N's conv interleaves img N+1's
            # transpose chunks (keeps matmuls flowing through the PE) ---
            total = IMGS_PER_CORE * repeats
            X0 = emit_load(0)
            XT8_cur, chunks = make_intr_chunks(X0)
            for ch in chunks:
                ch()
            for img_rep in range(total):
                if img_rep + 1 < total:
                    Xn = emit_load((img_rep + 1) % IMGS_PER_CORE)
                    XT8_next, nchunks = make_intr_chunks(Xn)
                else:
                    XT8_next, nchunks = None, []
                emit_conv(img_rep % IMGS_PER_CORE, XT8_cur, nchunks)
                XT8_cur = XT8_next

    _split_multi_waits(nc)
    _PROGRAM_CACHE[key] = nc
    return nc


def _in_maps(x: np.ndarray, w8: np.ndarray, alpha_col: np.ndarray):
    maps = []
    for i in range(N_CORES):
        maps.append({
            "x": x[i * IMGS_PER_CORE:(i + 1) * IMGS_PER_CORE],
            "w8": w8,
            "alpha_col": alpha_col,
        })
    return maps


def _pack_inputs(x, w_fp):
    x = np.ascontiguousarray(x, dtype=np.float32)
    w8, alpha_col = _pack_weights(np.asarray(w_fp, dtype=np.float32))
    return _in_maps(x, w8, alpha_col)


def kernel(x: np.ndarray, w_fp: np.ndarray) -> np.ndarray:
    assert x.shape == (64, 128, 128, 64) and w_fp.shape == (3, 3, 64, 64)
    nc = _build_program()
    res = run_bass_kernel_spmd(nc, _pack_inputs(x, w_fp),
                               core_ids=list(range(N_CORES)))
    out = np.concatenate([r["out"] for r in res.results], axis=0)
    kernel.last_results = res
    return out


# revision 10
# speedup vs baseline: 2.8514x; 1.0499x over previous
"""Binary 3x3 conv v4: PE-mode transposes + fp8 DoubleRow matmuls.

out = alpha * (sign(x) conv sign(w)), NHWC, SAME padding.
Data-parallel over batch: each of the 8 cores handles 8 images.

Design: keep the SDMA fabric at the pure HBM floor (4 MiB read + 4 MiB
write per image) by doing BOTH layout transposes on the TensorEngine
(128x128 transpose-mode tiles pipeline at ~81-107 ns), and halving the
conv's PE time with fp8e4 DoubleRow matmuls (virtual K=256; the k-tile
pair strides 128 B through the channel-major tile, which the 3D rhs AP
expresses directly).

Per-core pipeline per image (measured ~289 us/core for 8 images; the
xbar-transpose baseline was 554 us; HBM roofline is ~187 us):
  1. SWDGE cast-DMA (fp32 HBM -> bf16 SBUF) row-major X [r=128, (pp,ci)]
     with 1 zeroed pad pixel each side; ACT Sign in place -> +-1 bf16
     (pads memset BEFORE sign since sign(0)=+1).
  2. PE transpose-mode, 65 tiles [128,128]: X tile c (pixels pp=2c,2c+1)
     -> PSUM bf16 [q=(wlo,ci), r], 8 tiles per bank; evicts to XT8 fp8
     split half DVE / half ACT so the chain is PE-bound.  Emitted as
     closures interleaved between the PREVIOUS image's matmul groups
     (transpose-mode doesn't count as PE-busy for HAM; a 65-tile stretch
     would re-throttle the PE clock to 1.2 GHz).
  3. fp8 DoubleRow conv: per 8-row x 64-pair output block, 3 matmuls
     (kh in 1,0,2), lhsT=[128,(t=2),128] w8[kh], rhs=[128,(t=2),(c'=64),
     (r<=8)] with t,c' both striding 128 fp8 (overlapping raw AP) ->
     psum fp32 [m=(par,co), n=r*64+c'].  Outputs w'=2c'+par; virtual
     K=256 at 75% utilization.
  4. DVE tensor_scalar_mul evict: psum * alpha[m%64] (fp32 [128,1] AP)
     -> fp16 O4 (alpha cannot live in fp8 weights: 3 mantissa bits).
  5. PE transpose-mode 16 tiles/grp: O4 -> PSUM fp16; evicts split
     ACT/DVE -> T2 fp16 pixel-major (psum n-layout makes the transposed
     partition map to a uniform HBM stride, keeping the store AP 3-dim).
  6. SWDGE cast-DMA store fp16 -> fp32 HBM (512 B HBM runs).
"""

import os
import sys

sys.path.insert(0, "/opt/trn_rl_repo")

import numpy as np
import ml_dtypes

import concourse.bass as bass
import concourse.mybir as mybir
from concourse.bass_types import AP
from concourse import masks
from concourse.tile import TileContext
from concourse.bass_utils import run_bass_kernel_spmd

N_CORES = 8
IMGS_PER_CORE = 8
H = W = 128
C = 64
ROW = W * C           # 8192 bf16 per image row (data)
XROW = ROW + 128      # 8320 = 65*128: 1 pad pixel (64ch) each side
NT = 65               # transpose tiles per image


def _split_multi_waits(nc):
    """Single-sync-wait walrus workaround (see baseline)."""
    n_new = 0
    for f in nc.m.functions:
        for bb in f.blocks:
            insts = bb.instructions
            if not any(
                i.sync_info is not None and len(i.sync_info.on_wait) > 1
                for i in insts
            ):
                continue
            new = []
            for inst in insts:
                si = inst.sync_info
                if si is not None and len(si.on_wait) > 1:
                    waits = list(si.on_wait)
                    for j, w in enumerate(waits[:-1]):
                        n_new += 1
                        new.append(mybir.InstNoOp(
                            name=f"{inst.name}-sw{j}",
                            engine=inst.engine,
                            bass_nofuse=True,
                            sync_info=mybir.SyncInfo(on_wait=[w], on_update=[]),
                        ))
                    si.on_wait.clear()
                    si.on_wait.append(waits[-1])
                new.append(inst)
            bb.instructions = new
    return n_new


def _pack_weights(w_fp: np.ndarray):
    """Host prep: fp8 sign weights for DoubleRow + fp32 alpha column.

    w8[kh][64*wlo+ci, t, m]: m<64 -> kw=2t+wlo (if <=2); m>=64 -> kw=
    2t+wlo-1 (if >=0); else 0.  alpha_col[m] = alpha[m % 64]."""
    alpha = np.mean(np.abs(w_fp), axis=(0, 1, 2)).astype(np.float32)  # (co,)
    s = np.where(w_fp >= 0, 1.0, -1.0).astype(np.float32)  # (kh,kw,ci,co)
    w8 = np.zeros((3, 128, 2, 128), np.float32)
    for kh in range(3):
        for wlo in range(2):
            for t in range(2):
                kw_e = 2 * t + wlo        # even outputs (par=0, m<64)
                if kw_e <= 2:
                    w8[kh, 64 * wlo:64 * wlo + 64, t, 0:64] = s[kh, kw_e]
                kw_o = 2 * t + wlo - 1    # odd outputs (par=1, m>=64)
                if kw_o >= 0:
                    w8[kh, 64 * wlo:64 * wlo + 64, t, 64:128] = s[kh, kw_o]
    w8 = w8.astype(ml_dtypes.float8_e4m3)
    alpha_col = np.tile(alpha, 2)[:, None].astype(np.float32)  # (128, 1)
    return np.ascontiguousarray(w8), np.ascontiguousarray(alpha_col)


_PROGRAM_CACHE = {}


def _build_program(repeats: int = 1, skip: tuple = ()):
    key = (repeats, tuple(sorted(skip)))
    if key in _PROGRAM_CACHE:
        return _PROGRAM_CACHE[key]
    skip = set(skip)

    f32 = mybir.dt.float32
    f16 = mybir.dt.float16
    bf16 = mybir.dt.bfloat16
    fp8 = mybir.dt.float8e4
    Copy = mybir.ActivationFunctionType.Copy
    DR = mybir.MatmulPerfMode.DoubleRow

    nc = bass.Bass()
    x_d = nc.dram_tensor("x", (IMGS_PER_CORE, H, W, C), f32, kind="ExternalInput")
    w8_d = nc.dram_tensor("w8", (3, 128, 2, 128), fp8, kind="ExternalInput")
    al_d = nc.dram_tensor("alpha_col", (128, 1), f32, kind="ExternalInput")
    out_d = nc.dram_tensor("out", (IMGS_PER_CORE, H, W, C), f32, kind="ExternalOutput")

    x_flat = x_d.rearrange("i h w c -> i (h w c)")      # [8, 1048576]
    out_flat = out_d.rearrange("i h w c -> i (h w c)")  # [8, 1048576]

    with TileContext(nc) as tc:
        with (
            tc.tile_pool(name="wpool", bufs=1) as wpool,
            tc.tile_pool(name="xpool", bufs=2) as xpool,
            tc.tile_pool(name="x8pool", bufs=2) as x8pool,
            tc.tile_pool(name="tps", bufs=2, space="PSUM") as tps,
            tc.tile_pool(name="cps", bufs=4, space="PSUM") as cps,
            tc.tile_pool(name="ops", bufs=2, space="PSUM") as ops,
            tc.tile_pool(name="opool", bufs=3) as opool,
            tc.tile_pool(name="tpool", bufs=3) as tpool,
        ):
            w8_sb = wpool.tile([128, 768], fp8)
            nc.sync.dma_start(
                out=w8_sb.rearrange("k (i t m) -> k i t m", i=3, t=2),
                in_=w8_d.rearrange("i k t m -> k i t m"))
            w8v = w8_sb.rearrange("k (i t m) -> k i t m", i=3, t=2)
            alpha_sb = wpool.tile([128, 1], f32)
            nc.sync.dma_start(out=alpha_sb[:], in_=al_d[:])
            identb = wpool.tile([128, 128], bf16)
            masks.make_identity(nc, identb[:])
            identh = wpool.tile([128, 128], f16)
            masks.make_identity(nc, identh[:])
            if skip:
                XCONST = wpool.tile([128, XROW], bf16)
                nc.vector.memset(XCONST[:, :], 0.0)
                X8CONST = wpool.tile([128, XROW], fp8)
                nc.vector.memset(X8CONST[:, :], 0.0)
                O4CONST = wpool.tile([128, 2048], f16)
                nc.vector.memset(O4CONST[:, :], 0.0)
                T2CONST = wpool.tile([128, 2048], f16)
                nc.vector.memset(T2CONST[:, :], 0.0)

            def emit_load(img):
                """1. load (cast fp32 -> bf16), memset pads, ACT Sign in
                place -> +-1 bf16 with zero pads (sign runs BEFORE the
                transposes, so the transpose chain's evicts are plain
                copies splittable across DVE and ACT)."""
                if "cast" in skip:
                    return XCONST
                X = xpool.tile([128, XROW], bf16, tag="X")
                nc.vector.memset(X[:, 0:64], 0.0)
                nc.vector.memset(X[:, 64 + ROW:], 0.0)
                nc.gpsimd.dma_start(
                    out=X[:, 64:64 + ROW],
                    in_=x_flat[img].rearrange("(h i) -> h i", h=128),
                )
                nc.scalar.sign(X[:, 64:64 + ROW], X[:, 64:64 + ROW])
                return X

            def make_intr_chunks(X):
                """2+3. closures for bf16 PE-transpose chunks (8 tiles per
                PSUM bank) + copy-evicts (bf16 psum -> fp8 SBUF) alternating
                DVE/ACT.

                Returned as closures so the caller can interleave them with
                the previous image's matmuls (transpose-mode doesn't count
                as PE-busy for HAM; a 65-tile stretch would re-throttle the
                clock to 1.2 GHz)."""
                if "intr" in skip:
                    return X8CONST, []
                XT8 = x8pool.tile([128, XROW], fp8, tag="XT8")

                def chunk(j):
                    def go():
                        c1 = min(8 * j + 8, NT)
                        n = (c1 - 8 * j) * 128
                        pst = tps.tile([128, 1024], bf16, tag="pst")
                        for i, c in enumerate(range(8 * j, c1)):
                            nc.tensor.transpose(
                                pst[:, 128 * i:128 * i + 128],
                                X[:, 128 * c:128 * c + 128], identb[:])
                        # split the evict across DVE and ACT so the chain's
                        # per-chunk period is PE-bound, not evict-bound
                        h = min(512, n)
                        nc.vector.tensor_copy(
                            XT8[:, 1024 * j:1024 * j + h], pst[:, 0:h])
                        if n > h:
                            nc.scalar.activation(
                                out=XT8[:, 1024 * j + h:1024 * j + n],
                                in_=pst[:, h:n], func=Copy)
                    return go

                return XT8, [chunk(j) for j in range(9)]

            def emit_conv(img, XT8, next_chunks):
                """4-7. conv + out-transpose + store, interleaving the next
                image's transpose chunks between groups."""
                xt8_ap = XT8[:]
                pstride = xt8_ap.ap[0][0]

                def dr_rhs(free_off, cnt):
                    return AP(xt8_ap.tensor, xt8_ap.offset + free_off,
                              [[pstride, 128], [128, 2], [128, 64], [1, cnt]])

                # psum memory layout n = r*64 + c' (c' innermost) so the
                # out-transpose q = (rlo, c') maps to a uniform HBM stride
                # (3-dim store AP; >3 dims unbalanceable).
                for grp in range(4):
                    # interleave 2-3 next-image transpose chunks per grp
                    lo = (9 * grp) // 4
                    hi = (9 * (grp + 1)) // 4
                    for j in range(lo, min(hi, len(next_chunks))):
                        next_chunks[j]()
                    if "mm" not in skip:
                        O4 = opool.tile([128, 2048], f16, tag="O4")
                        for blk in range(4):
                            h0 = grp * 32 + blk * 8
                            psum = cps.tile([128, 512], f32, tag="ps")
                            psv = psum.rearrange("m (r c) -> m c r", c=64)
                            for idx, kh in enumerate((1, 0, 2)):
                                rbase = h0 + kh - 1
                                r_lo = max(0, -rbase)
                                r_hi = min(8, 128 - rbase)
                                rhs = dr_rhs(rbase + r_lo, r_hi - r_lo)
                                nc.tensor.matmul(
                                    psv[:, :, r_lo:r_hi], w8v[:, kh], rhs,
                                    start=(idx == 0), stop=(idx == 2),
                                    perf_mode=DR)
                            nc.vector.tensor_scalar_mul(
                                O4[:, 512 * blk:512 * blk + 512],
                                psum[:], alpha_sb[:, 0:1])
                    else:
                        O4 = O4CONST
                    # --- 6. out-transpose 16 tiles + ACT evict ---
                    if "otr" not in skip:
                        T2 = tpool.tile([128, 2048], f16, tag="T2")
                        for half in range(2):
                            pso = ops.tile([128, 1024], f16, tag="pso")
                            for i in range(8):
                                m16 = 8 * half + i
                                nc.tensor.transpose(
                                    pso[:, 128 * i:128 * i + 128],
                                    O4[:, 128 * m16:128 * m16 + 128],
                                    identh[:])
                            dst0 = 1024 * half
                            nc.scalar.activation(
                                out=T2[:, dst0:dst0 + 512],
                                in_=pso[:, 0:512], func=Copy)
                            nc.vector.tensor_copy(
                                T2[:, dst0 + 512:dst0 + 1024],
                                pso[:, 512:1024])
                    else:
                        T2 = T2CONST
                    # --- 7. cast-store fp16 -> fp32 (512 B HBM runs) ---
                    if "store" not in skip:
                        dst = out_flat[img, grp * 262144:(grp + 1) * 262144]
                        nc.gpsimd.dma_start(
                            out=dst.rearrange("(m q p) -> q m p",
                                              m=16, q=128, p=128),
                            in_=T2.rearrange("q (m p) -> q m p", p=128),
                        )

            # --- software pipeline: img N's conv interleaves img N+1's
            # transpose chunks (keeps matmuls flowing through the PE) ---
            total = IMGS_PER_CORE * repeats
            X0 = emit_load(0)
            XT8_cur, chunks = make_intr_chunks(X0)
            for ch in chunks:
                ch()
            for img_rep in range(total):
                if img_rep + 1 < total:
                    Xn = emit_load((img_rep + 1) % IMGS_PER_CORE)
                    XT8_next, nchunks = make_intr_chunks(Xn)
                else:
                    XT8_next, nchunks = None, []
                emit_conv(img_rep % IMGS_PER_CORE, XT8_cur, nchunks)
                XT8_cur = XT8_next

    _split_multi_waits(nc)
    _PROGRAM_CACHE[key] = nc
    return nc


def _in_maps(x: np.ndarray, w8: np.ndarray, alpha_col: np.ndarray):
    maps = []
    for i in range(N_CORES):
        maps.append({
            "x": x[i * IMGS_PER_CORE:(i + 1) * IMGS_PER_CORE],
            "w8": w8,
            "alpha_col": alpha_col,
        })
    return maps


def _pack_inputs(x, w_fp):
    x = np.ascontiguousarray(x, dtype=np.float32)
    w8, alpha_col = _pack_weights(np.asarray(w_fp, dtype=np.float32))
    return _in_maps(x, w8, alpha_col)


def kernel(x: np.ndarray, w_fp: np.ndarray) -> np.ndarray:
    assert x.shape == (64, 128, 128, 64) and w_fp.shape == (3, 3, 64, 64)
    nc = _build_program()
    res = run_bass_kernel_spmd(nc, _pack_inputs(x, w_fp),
                               core_ids=list(range(N_CORES)))
    out = np.concatenate([r["out"] for r in res.results], axis=0)
    kernel.last_results = res
    return out
